# revision 21
# baseline (speedup 1.0000x reference)
"""Trainium2 Bass kernel for nn_BasicBlock (conv-SE-prune-BN residual block).

Data-parallel over batch across 8 NeuronCores. Per core (B_loc = 1024),
processed in groups of 3 six-sample conv tiles (18 samples):
  P0  : stream x, per-(channel,sample) spatial mean (pooling)
  fc  : fc1-relu-fc2-sigmoid gates (tiny PE matmuls)
  AG  : AllGather all B*C gates; global-threshold bisection (22 fixed
        count-below-T iterations on ACT, interleaved with conv1)
  conv1: 3x3 conv = 3 K=128/M=128 bf16 matmuls per tile; K halves are
        channels + a flat-shifted duplicate (one contiguous SBUF DMA);
        M halves are two accumulators, combined via an SBUF bounce
  P3a : out1 * relu(gate - T), BN1 partial stats
  AR2 : AllReduce BN1 stats -> affine coefs
  P3b : bn1-affine+relu -> conv2 -> BN2 partial stats
  AR3 : AllReduce BN2 stats
  P5  : bn2-affine + residual + relu -> out

kernel(**inputs) takes the FULL inputs and returns the FULL output.
"""
import numpy as np

import concourse.bacc as bacc
import concourse.bass as bass
import concourse.mybir as mybir
import concourse.tile as tile

F32 = mybir.dt.float32
BF16 = mybir.dt.bfloat16
I32 = mybir.dt.int32
AF = mybir.ActivationFunctionType
ALU = mybir.AluOpType
AX = mybir.AxisListType

C = 64
HW = 64          # 8*8 spatial
TILE_B = 6
GROUP_T = 3      # conv tiles per group (shared psum tensor / DMAs)
PRUNE_RATE = 0.2
EPS = 1e-5
BISECT_ITERS = 22
PADSZ = 10 * 9   # padded sample size


def _groups(b_loc):
    """[(b0, [nb per tile])]; all but possibly the last have full tiles."""
    tiles = []
    b0 = 0
    while b0 < b_loc:
        nb = min(TILE_B, b_loc - b0)
        tiles.append((b0, nb))
        b0 += nb
    out = []
    i = 0
    while i < len(tiles):
        grp = tiles[i:i + GROUP_T]
        out.append((grp[0][0], [nb for (_, nb) in grp]))
        i += GROUP_T
    return out


def _transpose64(nc, dst_ap, src_ap):
    # full 64x64 transpose from per-32-block vector.transpose
    for i in (0, 32):
        for j in (0, 32):
            nc.vector.transpose(out=dst_ap[j:j + 32, i:i + 32],
                                in_=src_ap[i:i + 32, j:j + 32])


def build_nc(n_cores, b_loc):
    B_glob = n_cores * b_loc
    k_prune = int(PRUNE_RATE * B_glob * C)
    G = (b_loc * C * n_cores) // 128
    # sum of sign(T-g) = 2*count_less - N ; count_less <= k <=> sum <= 2k-N
    D0s = float(2 * k_prune - B_glob * C)
    N1 = float(B_glob * HW)
    groups = _groups(b_loc)
    NG = len(groups)
    rg = [list(range(n_cores))]
    GB = GROUP_T * TILE_B

    nc = bacc.Bacc("TRN2", target_bir_lowering=False, debug=False,
                   enable_asserts=True, num_devices=n_cores)

    x_in = nc.dram_tensor("x", [b_loc, C, 8, 8], F32, kind="ExternalInput")
    w1_in = nc.dram_tensor("conv1_w", [C, C, 3, 3], F32, kind="ExternalInput")
    w2_in = nc.dram_tensor("conv2_w", [C, C, 3, 3], F32, kind="ExternalInput")
    fc1w_in = nc.dram_tensor("fc1_w", [16, C], F32, kind="ExternalInput")
    fc1b_in = nc.dram_tensor("fc1_b", [16], F32, kind="ExternalInput")
    fc2w_in = nc.dram_tensor("fc2_w", [C, 16], F32, kind="ExternalInput")
    fc2b_in = nc.dram_tensor("fc2_b", [C], F32, kind="ExternalInput")
    bn1g_in = nc.dram_tensor("bn1_g", [C], F32, kind="ExternalInput")
    bn1b_in = nc.dram_tensor("bn1_b", [C], F32, kind="ExternalInput")
    bn2g_in = nc.dram_tensor("bn2_g", [C], F32, kind="ExternalInput")
    bn2b_in = nc.dram_tensor("bn2_b", [C], F32, kind="ExternalInput")
    out_d = nc.dram_tensor("out", [b_loc, C, 8, 8], F32, kind="ExternalOutput")

    with tile.TileContext(nc) as tc:
        with (
            tc.tile_pool(name="persist", bufs=1) as pp,
            tc.tile_pool(name="stg", bufs=2) as stgp,
            tc.tile_pool(name="pads", bufs=1) as padp,
            tc.tile_pool(name="small", bufs=2) as smallp,
            tc.tile_pool(name="prer", bufs=2) as prep,
            tc.tile_pool(name="dram", bufs=1, space="DRAM") as dramp,
        ):
            # early dummy collective absorbs cross-core start skew
            bar_sb = pp.tile([1, 1], F32, tag="bar_sb")
            bar_in = dramp.tile([1, 1], F32, tag="bar_in")
            bar_out = dramp.tile([1, 1], F32, tag="bar_out")
            nc.vector.memset(bar_sb[:], 0)
            nc.sync.dma_start(bar_in[:], bar_sb[:])
            nc.gpsimd.collective_compute(
                "AllReduce", ALU.add, replica_groups=rg,
                ins=[bar_in.opt()], outs=[bar_out.opt()])

            # ---------------- constants / weights prep ----------------
            w1_sb = pp.tile([C, C, 3, 3], F32, tag="w1")
            w2_sb = pp.tile([C, C, 3, 3], F32, tag="w2")
            nc.sync.dma_start(w1_sb[:], w1_in[:])
            nc.sync.dma_start(w2_sb[:], w2_in[:])
            lhs1, lhs2 = [], []
            for (wsb, lst, nm) in ((w1_sb, lhs1, "l1"), (w2_sb, lhs2, "l2")):
                for dy in range(3):
                    lt = pp.tile([128, 128], BF16, tag=f"{nm}_{dy}")
                    nc.vector.memset(lt[:], 0)
                    for (kp, mp, dx) in ((0, 0, 0), (64, 0, 1), (64, 64, 2)):
                        tp = smallp.tile([C, C], F32, tag="wtr")
                        _transpose64(nc, tp[:], wsb[:, :, dy, dx])
                        nc.vector.tensor_copy(lt[kp:kp + 64, mp:mp + 64], tp[:])
                    lst.append(lt)

            fc1T = pp.tile([C, C], F32, tag="fc1T")   # [64, 16] used
            fc2T = pp.tile([C, C], F32, tag="fc2T")   # [16, 64] used
            for (w_in_, shape, dstT) in ((fc1w_in, (16, C), fc1T),
                                         (fc2w_in, (C, 16), fc2T)):
                tmp = smallp.tile([C, C], F32, tag="fctmp")
                nc.vector.memset(tmp[:], 0)
                nc.sync.dma_start(tmp[0:shape[0], 0:shape[1]], w_in_[:])
                _transpose64(nc, dstT[:], tmp[:])

            vecs = pp.tile([C, 8], F32, tag="vecs")
            # cols: 0=fc2_b 1=bn1_g 2=bn1_b 3=bn2_g 4=bn2_b
            nc.sync.dma_start(vecs[:, 0:1], fc2b_in[:].unsqueeze(1))
            nc.sync.dma_start(vecs[:, 1:2], bn1g_in[:].unsqueeze(1))
            nc.sync.dma_start(vecs[:, 2:3], bn1b_in[:].unsqueeze(1))
            nc.sync.dma_start(vecs[:, 3:4], bn2g_in[:].unsqueeze(1))
            nc.sync.dma_start(vecs[:, 4:5], bn2b_in[:].unsqueeze(1))
            fc1b = pp.tile([16, 1], F32, tag="fc1b")
            nc.sync.dma_start(fc1b[:], fc1b_in[:].unsqueeze(1))

            onesKM = pp.tile([128, 128], BF16, tag="ones")
            nc.vector.memset(onesKM[:], 1.0)
            eps_t = pp.tile([C, 1], F32, tag="eps")
            nc.vector.memset(eps_t[:], EPS)

            # padded-input rings [128, slot, GB, 10, 9]; borders stay 0.
            NSLOT = 2
            xpad = padp.tile([128, NSLOT, GB, 10, 9], BF16, tag="xpad")
            ypad = padp.tile([128, NSLOT, GB, 10, 9], BF16, tag="ypad")
            nc.vector.memset(xpad[:], 0)
            nc.vector.memset(ypad[:], 0)
            xpad_f = xpad[:].rearrange("p s b r w -> p s (b r w)")
            ypad_f = ypad[:].rearrange("p s b r w -> p s (b r w)")

            NT = sum(len(nbs) for (_, nbs) in groups)
            R = pp.tile([C, NT, TILE_B, HW], BF16, tag="R")
            pooled = pp.tile([C, b_loc], F32, tag="pooled")
            gates = pp.tile([C, b_loc], F32, tag="gates")
            # stats sections of NT per-tile cols: S1, Q1, S2, Q2 (merged
            # group ops write their sum into the group's first tile column)
            stats = pp.tile([C, 4 * NT], F32, tag="stats")
            nc.vector.memset(stats[:], 0)
            sq_l = pp.tile([C, 4], F32, tag="sq_l")
            cf1 = pp.tile([C, 2], F32, tag="cf1")
            cf2 = pp.tile([C, 2], F32, tag="cf2")
            scratch = pp.tile([C, 8], F32, tag="scratch")

            # dram bounce buffers for collectives
            ag_in = dramp.tile([C, b_loc], F32, tag="ag_in")
            ag_out = dramp.tile([n_cores, C, b_loc], F32, tag="ag_out")
            ar_in = dramp.tile([C, 2], F32, tag="ar_in")
            ar_out = dramp.tile([C, 2], F32, tag="ar_out")
            ar2_in = dramp.tile([C, 2], F32, tag="ar2_in")
            ar2_out = dramp.tile([C, 2], F32, tag="ar2_out")

            def x_src(b0, ns):
                return x_in[b0:b0 + ns].transpose([1, 0, 2, 3])

            # ---------------- P0: pooling pass ----------------
            for (b0, nbs) in groups:
                ns = sum(nbs)
                stg = stgp.tile([C, GB, 8, 8], F32, tag="stg")
                nc.sync.dma_start(stg[:, 0:ns], x_src(b0, ns))
                nc.vector.tensor_reduce(out=pooled[:, b0:b0 + ns],
                                        in_=stg[:, 0:ns], axis=AX.XY,
                                        op=ALU.add)

            gatap_cm = tc.tile_pool(name="gatap", bufs=1)
            gatap = gatap_cm.__enter__()
            gata = gatap.tile([128, G], F32, tag="gata")
            cjunk = gatap.tile([128, G], BF16, tag="cjunk")

            # ---------- gates: fc1 relu fc2 sigmoid (scoped psum) ----------
            with tc.tile_pool(name="ps_fc", bufs=2, space="PSUM") as psm:
                # z1 is overlaid on pooled[0:16] (each chunk read before write)
                for j in range(0, b_loc, 512):
                    e = min(j + 512, b_loc)
                    zp = psm.tile([C, 512], F32, tag="zfc")
                    nc.tensor.matmul(zp[0:16, 0:e - j], fc1T[:, 0:16],
                                     pooled[:, j:e], start=True, stop=True)
                    nc.scalar.activation(pooled[0:16, j:e], zp[0:16, 0:e - j],
                                         AF.Relu, scale=1.0 / HW, bias=fc1b[:])
                for j in range(0, b_loc, 512):
                    e = min(j + 512, b_loc)
                    zp = psm.tile([C, 512], F32, tag="zfc")
                    nc.tensor.matmul(zp[:, 0:e - j], fc2T[0:16, :],
                                     pooled[0:16, j:e], start=True, stop=True)
                    nc.scalar.activation(gates[:, j:e], zp[:, 0:e - j],
                                         AF.Sigmoid, bias=vecs[:, 0:1])

            # allgather gates, load as [128, G]
            nc.sync.dma_start(ag_in[:], gates[:])
            nc.gpsimd.collective_compute(
                "AllGather", ALU.bypass, replica_groups=rg,
                ins=[ag_in.opt()], outs=[ag_out.opt()])
            nc.sync.dma_start(
                gata[:], ag_out[:].rearrange("n c b -> (n c b)")
                .rearrange("(p g) -> p g", p=128))

            psc_cm = tc.tile_pool(name="ps_conv", bufs=2, space="PSUM")
            psc = psc_cm.__enter__()
            psb_cm = tc.tile_pool(name="ps_bis", bufs=2, space="PSUM")
            psb = psb_cm.__enter__()

            # ---------------- bisection machinery ----------------
            lh = pp.tile([128, 2], F32, tag="lh")
            Tt = pp.tile([128, 1], F32, tag="Tt")
            nc.vector.memset(lh[:, 0:1], 0.0)
            nc.vector.memset(lh[:, 1:2], 1.0)

            bis_at = {}
            bstart = NG - 2 - 2 * (BISECT_ITERS - 1)
            if bstart >= 1:
                for j in range(BISECT_ITERS):
                    bis_at[bstart + 2 * j] = 1
            else:
                bis_at[max(0, NG - 2)] = BISECT_ITERS

            def bisect_iter():
                tj = smallp.tile([128, 2], F32, tag="bj")
                nc.vector.tensor_scalar(out=tj[:], in0=lh[:], scalar1=0.5,
                                        scalar2=None, op0=ALU.mult,
                                        op1=ALU.add, accum_out=Tt[:])
                cnt = smallp.tile([128, 1], F32, tag="bcnt")
                nc.scalar.activation(cjunk[:], gata[:], AF.Sign,
                                     scale=-1.0, bias=Tt[:], accum_out=cnt[:])
                cntb = smallp.tile([128, 1], BF16, tag="bcntb")
                nc.vector.tensor_copy(cntb[:], cnt[:])
                psum_c = psb.tile([128, 1], F32, tag="bps")
                nc.tensor.matmul(psum_c[:], onesKM[:], cntb[:],
                                 start=True, stop=True)
                m_le = smallp.tile([128, 1], I32, tag="bmle")
                m_gt = smallp.tile([128, 1], I32, tag="bmgt")
                nc.vector.tensor_scalar(out=m_le[:], in0=psum_c[:],
                                        scalar1=D0s, scalar2=None,
                                        op0=ALU.is_le)
                nc.vector.tensor_scalar(out=m_gt[:], in0=psum_c[:],
                                        scalar1=D0s, scalar2=None,
                                        op0=ALU.is_gt)
                nc.vector.copy_predicated(out=lh[:, 0:1], mask=m_le[:],
                                          data=Tt[:])
                nc.vector.copy_predicated(out=lh[:, 1:2], mask=m_gt[:],
                                          data=Tt[:])

            def ps_a_view(ps):
                # A-half [64, t, b, 8, 0:8] view of grouped psum (full groups)
                return ps[0:64, :].rearrange(
                    "p (t x) -> p t x", t=GROUP_T, x=512)[:, :, 0:432] \
                    .rearrange("p t (b r w) -> p t b r w",
                               b=TILE_B, r=8, w=9)[:, :, :, :, 0:8]

            def ps_b_view(ps):
                return ps[64:128, :].rearrange(
                    "p (t x) -> p t x", t=GROUP_T, x=512)[:, :, 0:432] \
                    .rearrange("p t (b r w) -> p t b r w",
                               b=TILE_B, r=8, w=9)[:, :, :, :, 1:9]

            def conv_group(gi, nbs, lhs, pad):
                """3*GROUP_T matmuls (dy-major); B-half bounced to parts 0:64."""
                slot = gi % NSLOT
                ful = all(nb == TILE_B for nb in nbs)
                ps = psc.tile([128, GROUP_T * 512], F32, tag="cps")
                for dy in range(3):
                    for (t, nb) in enumerate(nbs):
                        nc.tensor.matmul(
                            ps[:, 512 * t:512 * t + nb * 72].rearrange(
                                "p (b r w) -> p b r w", b=nb, r=8, w=9),
                            lhs[dy][:],
                            pad[:, slot, TILE_B * t:TILE_B * t + nb,
                                dy:dy + 8, :],
                            start=(dy == 0), stop=(dy == 2))
                cmb = prep.tile([128, GROUP_T, TILE_B, 8, 8], BF16, tag="cmb")
                for (t, nb) in enumerate(nbs):
                    src = ps[64:128, 512 * t:512 * t + nb * 72].rearrange(
                        "p (b r w) -> p b r w", b=nb, r=8, w=9)[:, :, :, 1:9]
                    if (gi + t) % 2 == 0:
                        nc.scalar.copy(cmb[64:128, t, 0:nb], src)
                    else:
                        nc.vector.tensor_copy(cmb[64:128, t, 0:nb], src)
                if ful:
                    nc.sync.dma_start(cmb[0:64], cmb[64:128])
                else:
                    for (t, nb) in enumerate(nbs):
                        nc.sync.dma_start(cmb[0:64, t, 0:nb],
                                          cmb[64:128, t, 0:nb])
                return ps, cmb

            # ---------------- conv1 + interleaved bisection ----------------
            for (gi, (b0, nbs)) in enumerate(groups):
                slot = gi % NSLOT
                ns = sum(nbs)
                ful = all(nb == TILE_B for nb in nbs)
                stg = stgp.tile([C, GB, 8, 8], F32, tag="stg")
                nc.sync.dma_start(stg[:, 0:ns], x_src(b0, ns))
                if ful:
                    nc.scalar.activation(
                        xpad[0:64, slot, :, 1:9, 1:9], stg[:], AF.Copy)
                else:
                    st = 0
                    for (t, nb) in enumerate(nbs):
                        nc.scalar.activation(
                            xpad[0:64, slot, TILE_B * t:TILE_B * t + nb,
                                 1:9, 1:9],
                            stg[:, st:st + nb], AF.Copy)
                        st += nb
                # flat shift-by-one duplicate (single contiguous run / part)
                nc.sync.dma_start(xpad_f[64:128, slot, 0:GB * PADSZ - 1],
                                  xpad_f[0:64, slot, 1:GB * PADSZ])
                ps, cmb = conv_group(gi, nbs, lhs1, xpad)
                for (t, nb) in enumerate(nbs):
                    nc.vector.tensor_tensor(
                        out=R[:, GROUP_T * gi + t, 0:nb].rearrange(
                            "p b (h w) -> p b h w", h=8, w=8),
                        in0=ps[0:64, 512 * t:512 * t + nb * 72].rearrange(
                            "p (b r w) -> p b r w", b=nb, r=8, w=9)
                        [:, :, :, 0:8],
                        in1=cmb[0:64, t, 0:nb], op=ALU.add)

                for _ in range(bis_at.get(gi, 0)):
                    bisect_iter()

            # final threshold -> -T
            tj = smallp.tile([128, 2], F32, tag="bj")
            nc.vector.tensor_scalar(out=tj[:], in0=lh[:], scalar1=0.5,
                                    scalar2=None, op0=ALU.mult,
                                    op1=ALU.add, accum_out=Tt[:])
            negT = pp.tile([128, 1], F32, tag="negT")
            nc.vector.tensor_scalar(out=negT[:], in0=Tt[:], scalar1=-1.0,
                                    scalar2=None, op0=ALU.mult)
            gatap_cm.__exit__(None, None, None)

            # ---------------- P3a: gate application + BN1 stats ----------------
            nc.scalar.activation(gates[:], gates[:], AF.Relu,
                                 bias=negT[0:64, :])
            sep = gates
            for (gi, (b0, nbs)) in enumerate(groups):
                ns = sum(nbs)
                if all(nb == TILE_B for nb in nbs):
                    rsl = R[:, GROUP_T * gi:GROUP_T * (gi + 1)].rearrange(
                        "p t b q -> p (t b) q")
                    sep_b = sep[:, b0:b0 + ns].unsqueeze(2).broadcast_to(
                        (C, ns, HW))
                    ti0 = GROUP_T * gi
                    nc.vector.scalar_tensor_tensor(
                        out=rsl, in0=rsl, scalar=1.0, in1=sep_b,
                        op0=ALU.mult, op1=ALU.mult,
                        accum_out=stats[:, ti0:ti0 + 1])
                    sqj = prep.tile([C, GB, HW], F32, tag="pre")
                    nc.scalar.activation(
                        sqj[:].rearrange("p b q -> p (b q)"),
                        rsl.rearrange("p b q -> p (b q)"), AF.Square,
                        accum_out=stats[:, NT + ti0:NT + ti0 + 1])
                else:
                    st = 0
                    for (t, nb) in enumerate(nbs):
                        ti = GROUP_T * gi + t
                        rsl = R[:, ti, 0:nb]
                        sep_b = sep[:, b0 + st:b0 + st + nb].unsqueeze(
                            2).broadcast_to((C, nb, HW))
                        nc.vector.scalar_tensor_tensor(
                            out=rsl, in0=rsl, scalar=1.0, in1=sep_b,
                            op0=ALU.mult, op1=ALU.mult,
                            accum_out=stats[:, ti:ti + 1])
                        sqj = prep.tile([C, GB, HW], F32, tag="pre")
                        nc.scalar.activation(
                            sqj[:, 0:nb].rearrange("p b q -> p (b q)"),
                            rsl.rearrange("p b q -> p (b q)"), AF.Square,
                            accum_out=stats[:, NT + ti:NT + ti + 1])
                        st += nb

            def stats_allreduce(c0, arin, arout, cf, gcol, bcol):
                nc.vector.tensor_reduce(
                    out=sq_l[:, 0:1], in_=stats[:, c0 * NT:(c0 + 1) * NT],
                    axis=AX.X, op=ALU.add)
                nc.vector.tensor_reduce(
                    out=sq_l[:, 1:2],
                    in_=stats[:, (c0 + 1) * NT:(c0 + 2) * NT],
                    axis=AX.X, op=ALU.add)
                nc.sync.dma_start(arin[:], sq_l[:, 0:2])
                nc.gpsimd.collective_compute(
                    "AllReduce", ALU.add, replica_groups=rg,
                    ins=[arin.opt()], outs=[arout.opt()])
                sq_g = smallp.tile([C, 2], F32, tag="sqg")
                nc.sync.dma_start(sq_g[:], arout[:])
                # scratch cols: 0=mean 1=E[x^2] 2=-var 3=sd 4=isd
                nc.vector.tensor_scalar(out=scratch[:, 0:2], in0=sq_g[:],
                                        scalar1=1.0 / N1, scalar2=None,
                                        op0=ALU.mult)
                nc.vector.scalar_tensor_tensor(
                    out=scratch[:, 2:3], in0=scratch[:, 0:1],
                    scalar=scratch[:, 0:1], in1=scratch[:, 1:2],
                    op0=ALU.mult, op1=ALU.subtract)
                nc.scalar.activation(scratch[:, 3:4], scratch[:, 2:3],
                                     AF.Sqrt, scale=-1.0, bias=eps_t[:])
                nc.vector.reciprocal(scratch[:, 4:5], scratch[:, 3:4])
                nc.vector.tensor_tensor(out=cf[:, 0:1],
                                        in0=vecs[:, gcol:gcol + 1],
                                        in1=scratch[:, 4:5], op=ALU.mult)
                nc.vector.scalar_tensor_tensor(
                    out=cf[:, 1:2], in0=scratch[:, 0:1],
                    scalar=cf[:, 0:1], in1=vecs[:, bcol:bcol + 1],
                    op0=ALU.mult, op1=ALU.subtract)
                nc.vector.tensor_scalar(out=cf[:, 1:2], in0=cf[:, 1:2],
                                        scalar1=-1.0, scalar2=None,
                                        op0=ALU.mult)

            stats_allreduce(0, ar_in, ar_out, cf1, 1, 2)

            # ------------- P3b: bn1+relu -> conv2 -> BN2 stats -------------
            for (gi, (b0, nbs)) in enumerate(groups):
                slot = gi % NSLOT
                ful = all(nb == TILE_B for nb in nbs)
                if ful:
                    nc.scalar.activation(
                        ypad[0:64, slot, :, 1:9, 1:9],
                        R[:, GROUP_T * gi:GROUP_T * (gi + 1)].rearrange(
                            "p t b (h w) -> p (t b) h w", h=8, w=8),
                        AF.Relu, scale=cf1[:, 0:1], bias=cf1[:, 1:2])
                else:
                    for (t, nb) in enumerate(nbs):
                        nc.scalar.activation(
                            ypad[0:64, slot, TILE_B * t:TILE_B * t + nb,
                                 1:9, 1:9],
                            R[:, GROUP_T * gi + t, 0:nb].rearrange(
                                "p b (h w) -> p b h w", h=8, w=8),
                            AF.Relu, scale=cf1[:, 0:1], bias=cf1[:, 1:2])
                nc.sync.dma_start(ypad_f[64:128, slot, 0:GB * PADSZ - 1],
                                  ypad_f[0:64, slot, 1:GB * PADSZ])
                ps, cmb = conv_group(gi, nbs, lhs2, ypad)
                for (t, nb) in enumerate(nbs):
                    ti = GROUP_T * gi + t
                    rsl = R[:, ti, 0:nb]
                    nc.vector.scalar_tensor_tensor(
                        out=rsl.rearrange("p b (h w) -> p b h w", h=8, w=8),
                        in0=ps[0:64, 512 * t:512 * t + nb * 72].rearrange(
                            "p (b r w) -> p b r w", b=nb, r=8, w=9)
                        [:, :, :, 0:8],
                        scalar=1.0, in1=cmb[0:64, t, 0:nb],
                        op0=ALU.mult, op1=ALU.add,
                        accum_out=stats[:, 2 * NT + ti:2 * NT + ti + 1])
                if ful:
                    ti0 = GROUP_T * gi
                    sqj = prep.tile([C, GB, HW], F32, tag="pre")
                    rfl = R[:, GROUP_T * gi:GROUP_T * (gi + 1)].rearrange(
                        "p t b q -> p (t b q)")
                    if gi % 2 == 1:
                        nc.scalar.activation(
                            sqj[:].rearrange("p b q -> p (b q)"), rfl,
                            AF.Square,
                            accum_out=stats[:, 3 * NT + ti0:3 * NT + ti0 + 1])
                    else:
                        nc.vector.scalar_tensor_tensor(
                            out=sqj[:].rearrange("p b q -> p (b q)"), in0=rfl,
                            scalar=1.0, in1=rfl, op0=ALU.mult, op1=ALU.mult,
                            accum_out=stats[:, 3 * NT + ti0:3 * NT + ti0 + 1])
                else:
                    for (t, nb) in enumerate(nbs):
                        ti = GROUP_T * gi + t
                        rsl = R[:, ti, 0:nb]
                        sqj = prep.tile([C, GB, HW], F32, tag="pre")
                        nc.scalar.activation(
                            sqj[:, 0:nb].rearrange("p b q -> p (b q)"),
                            rsl.rearrange("p b q -> p (b q)"), AF.Square,
                            accum_out=stats[:, 3 * NT + ti:3 * NT + ti + 1])

            stats_allreduce(2, ar2_in, ar2_out, cf2, 3, 4)

            # ---------------- P5: bn2 + residual + relu -> out ----------------
            latep_cm = tc.tile_pool(name="latep", bufs=2)
            latep = latep_cm.__enter__()
            latep2_cm = tc.tile_pool(name="latep2", bufs=3)
            latep2 = latep2_cm.__enter__()
            for (gi, (b0, nbs)) in enumerate(groups):
                ns = sum(nbs)
                ful = all(nb == TILE_B for nb in nbs)
                stg = latep.tile([C, GB, 8, 8], F32, tag="stg5")
                nc.sync.dma_start(stg[:, 0:ns], x_src(b0, ns))
                pre = latep2.tile([C, GB, HW], F32, tag="pre5")
                if ful:
                    rsl = R[:, GROUP_T * gi:GROUP_T * (gi + 1)].rearrange(
                        "p t b q -> p (t b) q")
                    nc.vector.scalar_tensor_tensor(
                        out=pre[:], in0=rsl, scalar=cf2[:, 0:1],
                        in1=stg[:].rearrange("p b h w -> p b (h w)"),
                        op0=ALU.mult, op1=ALU.add)
                else:
                    st = 0
                    for (t, nb) in enumerate(nbs):
                        rsl = R[:, GROUP_T * gi + t, 0:nb]
                        nc.vector.scalar_tensor_tensor(
                            out=pre[:, st:st + nb], in0=rsl,
                            scalar=cf2[:, 0:1],
                            in1=stg[:, st:st + nb].rearrange(
                                "p b h w -> p b (h w)"),
                            op0=ALU.mult, op1=ALU.add)
                        st += nb
                nc.scalar.activation(pre[:, 0:ns], pre[:, 0:ns], AF.Relu,
                                     bias=cf2[:, 1:2])
                nc.sync.dma_start(
                    out_d[b0:b0 + ns].transpose([1, 0, 2, 3]),
                    pre[:, 0:ns].rearrange("p b (h w) -> p b h w", h=8, w=8))

            latep2_cm.__exit__(None, None, None)
            latep_cm.__exit__(None, None, None)
            psb_cm.__exit__(None, None, None)
            psc_cm.__exit__(None, None, None)

    nc.compile()
    return nc


_NC_CACHE = {}


def _get_nc(n_cores, b_loc):
    key = (n_cores, b_loc)
    if key not in _NC_CACHE:
        _NC_CACHE[key] = build_nc(n_cores, b_loc)
    return _NC_CACHE[key]


def kernel(**inputs):
    from concourse.bass_utils import run_bass_kernel_spmd

    x = np.asarray(inputs["x"], dtype=np.float32)
    B = x.shape[0]
    n_cores = 8
    b_loc = B // n_cores
    nc = _get_nc(n_cores, b_loc)

    weight_names = ["conv1_w", "conv2_w", "fc1_w", "fc1_b", "fc2_w", "fc2_b",
                    "bn1_g", "bn1_b", "bn2_g", "bn2_b"]
    in_maps = []
    for c in range(n_cores):
        m = {"x": np.ascontiguousarray(x[c * b_loc:(c + 1) * b_loc])}
        for n in weight_names:
            m[n] = np.asarray(inputs[n], dtype=np.float32)
        in_maps.append(m)
    res = run_bass_kernel_spmd(nc, in_maps, core_ids=list(range(n_cores)))
    out = np.concatenate([res.results[c]["out"] for c in range(n_cores)],
                         axis=0)
    return out.astype(np.float32)


# revision 23
# speedup vs baseline: 1.2126x; 1.2126x over previous
"""Trainium2 Bass kernel for nn_BasicBlock (conv-SE-prune-BN residual block).

Data-parallel over batch across 8 NeuronCores. Per core (B_loc = 1024),
processed in groups of 3 six-sample conv tiles (18 samples):
  P0  : stream x, per-(channel,sample) spatial mean (pooling)
  fc  : fc1-relu-fc2-sigmoid gates (tiny PE matmuls)
  AG  : AllGather all B*C gates; global-threshold bisection (22 fixed
        count-below-T iterations on ACT, interleaved with conv1)
  conv1: 3x3 conv = 3 K=128/M=128 bf16 matmuls per tile; K halves are
        channels + a flat-shifted duplicate (one contiguous SBUF DMA);
        M halves are two accumulators, combined via an SBUF bounce
  P3a : out1 * relu(gate - T), BN1 partial stats
  AR2 : AllReduce BN1 stats -> affine coefs
  P3b : bn1-affine+relu -> conv2 -> BN2 partial stats
  AR3 : AllReduce BN2 stats
  P5  : bn2-affine + residual + relu -> out

kernel(**inputs) takes the FULL inputs and returns the FULL output.
"""
import numpy as np

import concourse.bacc as bacc
import concourse.bass as bass
import concourse.mybir as mybir
import concourse.tile as tile

F32 = mybir.dt.float32
BF16 = mybir.dt.bfloat16
I32 = mybir.dt.int32
AF = mybir.ActivationFunctionType
ALU = mybir.AluOpType
AX = mybir.AxisListType

C = 64
HW = 64          # 8*8 spatial
TILE_B = 6
GROUP_T = 3      # conv tiles per group (shared psum tensor / DMAs)
PRUNE_RATE = 0.2
EPS = 1e-5
BISECT_ITERS = 22
PADSZ = 10 * 9   # padded sample size


def _groups(b_loc):
    """[(b0, [nb per tile])]; all but possibly the last have full tiles."""
    tiles = []
    b0 = 0
    while b0 < b_loc:
        nb = min(TILE_B, b_loc - b0)
        tiles.append((b0, nb))
        b0 += nb
    out = []
    i = 0
    while i < len(tiles):
        grp = tiles[i:i + GROUP_T]
        out.append((grp[0][0], [nb for (_, nb) in grp]))
        i += GROUP_T
    return out


def _transpose64(nc, dst_ap, src_ap):
    # full 64x64 transpose from per-32-block vector.transpose
    for i in (0, 32):
        for j in (0, 32):
            nc.vector.transpose(out=dst_ap[j:j + 32, i:i + 32],
                                in_=src_ap[i:i + 32, j:j + 32])


def build_nc(n_cores, b_loc):
    B_glob = n_cores * b_loc
    k_prune = int(PRUNE_RATE * B_glob * C)
    G = (b_loc * C * n_cores) // 128
    # sum of sign(T-g) = 2*count_less - N ; count_less <= k <=> sum <= 2k-N
    D0s = float(2 * k_prune - B_glob * C)
    N1 = float(B_glob * HW)
    groups = _groups(b_loc)
    NG = len(groups)
    rg = [list(range(n_cores))]
    GB = GROUP_T * TILE_B

    nc = bacc.Bacc("TRN2", target_bir_lowering=False, debug=False,
                   enable_asserts=True, num_devices=n_cores)

    x_in = nc.dram_tensor("x", [b_loc, C, 8, 8], F32, kind="ExternalInput")
    w1_in = nc.dram_tensor("conv1_w", [C, C, 3, 3], F32, kind="ExternalInput")
    w2_in = nc.dram_tensor("conv2_w", [C, C, 3, 3], F32, kind="ExternalInput")
    fc1w_in = nc.dram_tensor("fc1_w", [16, C], F32, kind="ExternalInput")
    fc1b_in = nc.dram_tensor("fc1_b", [16], F32, kind="ExternalInput")
    fc2w_in = nc.dram_tensor("fc2_w", [C, 16], F32, kind="ExternalInput")
    fc2b_in = nc.dram_tensor("fc2_b", [C], F32, kind="ExternalInput")
    bn1g_in = nc.dram_tensor("bn1_g", [C], F32, kind="ExternalInput")
    bn1b_in = nc.dram_tensor("bn1_b", [C], F32, kind="ExternalInput")
    bn2g_in = nc.dram_tensor("bn2_g", [C], F32, kind="ExternalInput")
    bn2b_in = nc.dram_tensor("bn2_b", [C], F32, kind="ExternalInput")
    out_d = nc.dram_tensor("out", [b_loc, C, 8, 8], F32, kind="ExternalOutput")

    with tile.TileContext(nc) as tc:
        with (
            tc.tile_pool(name="persist", bufs=1) as pp,
            tc.tile_pool(name="stg", bufs=2) as stgp,
            tc.tile_pool(name="pads", bufs=1) as padp,
            tc.tile_pool(name="small", bufs=2) as smallp,
            tc.tile_pool(name="prer", bufs=2) as prep,
            tc.tile_pool(name="dram", bufs=1, space="DRAM") as dramp,
        ):
            # early dummy collective absorbs cross-core start skew
            bar_sb = pp.tile([1, 1], F32, tag="bar_sb")
            bar_in = dramp.tile([1, 1], F32, tag="bar_in")
            bar_out = dramp.tile([1, 1], F32, tag="bar_out")
            nc.vector.memset(bar_sb[:], 0)
            nc.sync.dma_start(bar_in[:], bar_sb[:])
            nc.gpsimd.collective_compute(
                "AllReduce", ALU.add, replica_groups=rg,
                ins=[bar_in.opt()], outs=[bar_out.opt()])

            # ---------------- constants / weights prep ----------------
            w1_sb = pp.tile([C, C, 3, 3], F32, tag="w1")
            w2_sb = pp.tile([C, C, 3, 3], F32, tag="w2")
            nc.sync.dma_start(w1_sb[:], w1_in[:])
            nc.sync.dma_start(w2_sb[:], w2_in[:])
            lhs1, lhs2 = [], []
            for (wsb, lst, nm) in ((w1_sb, lhs1, "l1"), (w2_sb, lhs2, "l2")):
                for dy in range(3):
                    lt = pp.tile([128, 128], BF16, tag=f"{nm}_{dy}")
                    nc.vector.memset(lt[:], 0)
                    for (kp, mp, dx) in ((0, 0, 0), (64, 0, 1), (64, 64, 2)):
                        tp = smallp.tile([C, C], F32, tag="wtr")
                        _transpose64(nc, tp[:], wsb[:, :, dy, dx])
                        nc.vector.tensor_copy(lt[kp:kp + 64, mp:mp + 64], tp[:])
                    lst.append(lt)

            fc1T = pp.tile([C, C], F32, tag="fc1T")   # [64, 16] used
            fc2T = pp.tile([C, C], F32, tag="fc2T")   # [16, 64] used
            for (w_in_, shape, dstT) in ((fc1w_in, (16, C), fc1T),
                                         (fc2w_in, (C, 16), fc2T)):
                tmp = smallp.tile([C, C], F32, tag="fctmp")
                nc.vector.memset(tmp[:], 0)
                nc.sync.dma_start(tmp[0:shape[0], 0:shape[1]], w_in_[:])
                _transpose64(nc, dstT[:], tmp[:])

            vecs = pp.tile([C, 8], F32, tag="vecs")
            # cols: 0=fc2_b 1=bn1_g 2=bn1_b 3=bn2_g 4=bn2_b
            nc.sync.dma_start(vecs[:, 0:1], fc2b_in[:].unsqueeze(1))
            nc.sync.dma_start(vecs[:, 1:2], bn1g_in[:].unsqueeze(1))
            nc.sync.dma_start(vecs[:, 2:3], bn1b_in[:].unsqueeze(1))
            nc.sync.dma_start(vecs[:, 3:4], bn2g_in[:].unsqueeze(1))
            nc.sync.dma_start(vecs[:, 4:5], bn2b_in[:].unsqueeze(1))
            fc1b = pp.tile([16, 1], F32, tag="fc1b")
            nc.sync.dma_start(fc1b[:], fc1b_in[:].unsqueeze(1))

            onesKM = pp.tile([128, 128], BF16, tag="ones")
            nc.vector.memset(onesKM[:], 1.0)
            eps_t = pp.tile([C, 1], F32, tag="eps")
            nc.vector.memset(eps_t[:], EPS)

            # padded-input rings [128, slot, GB, 10, 9]; borders stay 0.
            NSLOT = 2
            xpad = padp.tile([128, NSLOT, GB, 10, 9], BF16, tag="xpad")
            ypad = padp.tile([128, NSLOT, GB, 10, 9], BF16, tag="ypad")
            nc.vector.memset(xpad[:], 0)
            nc.vector.memset(ypad[:], 0)
            xpad_f = xpad[:].rearrange("p s b r w -> p s (b r w)")
            ypad_f = ypad[:].rearrange("p s b r w -> p s (b r w)")

            NT = sum(len(nbs) for (_, nbs) in groups)
            R = pp.tile([C, NT, TILE_B, HW], BF16, tag="R")
            pooled = pp.tile([C, b_loc], F32, tag="pooled")
            gates = pp.tile([C, b_loc], F32, tag="gates")
            # stats sections of NT per-tile cols: S1, Q1, S2, Q2 (merged
            # group ops write their sum into the group's first tile column)
            stats = pp.tile([C, 4 * NT], F32, tag="stats")
            nc.vector.memset(stats[:], 0)
            sq_l = pp.tile([C, 4], F32, tag="sq_l")
            cf1 = pp.tile([C, 2], F32, tag="cf1")
            cf2 = pp.tile([C, 2], F32, tag="cf2")
            scratch = pp.tile([C, 8], F32, tag="scratch")

            # dram bounce buffers for collectives
            ag_in = dramp.tile([C, b_loc], F32, tag="ag_in")
            ag_out = dramp.tile([n_cores, C, b_loc], F32, tag="ag_out")
            ar_in = dramp.tile([C, 2], F32, tag="ar_in")
            ar_out = dramp.tile([C, 2], F32, tag="ar_out")
            ar2_in = dramp.tile([C, 2], F32, tag="ar2_in")
            ar2_out = dramp.tile([C, 2], F32, tag="ar2_out")

            def x_src(b0, ns):
                return x_in[b0:b0 + ns].transpose([1, 0, 2, 3])

            # ---------------- P0: pooling pass ----------------
            # bigger chunks than the conv groups: the gates buffers are not
            # allocated yet, so borrow that SBUF for 36-sample staging
            p0_cm = tc.tile_pool(name="p0stg", bufs=2)
            p0p = p0_cm.__enter__()
            P0C = 2 * GB
            b0 = 0
            while b0 < b_loc:
                ns = min(P0C, b_loc - b0)
                stg = p0p.tile([C, P0C, 8, 8], F32, tag="stg0")
                nc.sync.dma_start(stg[:, 0:ns], x_src(b0, ns))
                nc.vector.tensor_reduce(out=pooled[:, b0:b0 + ns],
                                        in_=stg[:, 0:ns], axis=AX.XY,
                                        op=ALU.add)
                b0 += ns
            p0_cm.__exit__(None, None, None)

            gatap_cm = tc.tile_pool(name="gatap", bufs=1)
            gatap = gatap_cm.__enter__()
            gata = gatap.tile([128, G], F32, tag="gata")
            cjunk = gatap.tile([128, G], BF16, tag="cjunk")

            # ---------- gates: fc1 relu fc2 sigmoid (scoped psum) ----------
            with tc.tile_pool(name="ps_fc", bufs=2, space="PSUM") as psm:
                # z1 is overlaid on pooled[0:16] (each chunk read before write)
                for j in range(0, b_loc, 512):
                    e = min(j + 512, b_loc)
                    zp = psm.tile([C, 512], F32, tag="zfc")
                    nc.tensor.matmul(zp[0:16, 0:e - j], fc1T[:, 0:16],
                                     pooled[:, j:e], start=True, stop=True)
                    nc.scalar.activation(pooled[0:16, j:e], zp[0:16, 0:e - j],
                                         AF.Relu, scale=1.0 / HW, bias=fc1b[:])
                for j in range(0, b_loc, 512):
                    e = min(j + 512, b_loc)
                    zp = psm.tile([C, 512], F32, tag="zfc")
                    nc.tensor.matmul(zp[:, 0:e - j], fc2T[0:16, :],
                                     pooled[0:16, j:e], start=True, stop=True)
                    nc.scalar.activation(gates[:, j:e], zp[:, 0:e - j],
                                         AF.Sigmoid, bias=vecs[:, 0:1])

            # allgather gates, load as [128, G]
            nc.sync.dma_start(ag_in[:], gates[:])
            nc.gpsimd.collective_compute(
                "AllGather", ALU.bypass, replica_groups=rg,
                ins=[ag_in.opt()], outs=[ag_out.opt()])
            nc.sync.dma_start(
                gata[:], ag_out[:].rearrange("n c b -> (n c b)")
                .rearrange("(p g) -> p g", p=128))

            psc_cm = tc.tile_pool(name="ps_conv", bufs=2, space="PSUM")
            psc = psc_cm.__enter__()
            psb_cm = tc.tile_pool(name="ps_bis", bufs=2, space="PSUM")
            psb = psb_cm.__enter__()

            # ---------------- bisection machinery ----------------
            lh = pp.tile([128, 2], F32, tag="lh")
            Tt = pp.tile([128, 1], F32, tag="Tt")
            nc.vector.memset(lh[:, 0:1], 0.0)
            nc.vector.memset(lh[:, 1:2], 1.0)

            bis_at = {}
            bstart = NG - 2 - 2 * (BISECT_ITERS - 1)
            if bstart >= 1:
                for j in range(BISECT_ITERS):
                    bis_at[bstart + 2 * j] = 1
            else:
                bis_at[max(0, NG - 2)] = BISECT_ITERS

            def bisect_iter():
                tj = smallp.tile([128, 2], F32, tag="bj")
                nc.vector.tensor_scalar(out=tj[:], in0=lh[:], scalar1=0.5,
                                        scalar2=None, op0=ALU.mult,
                                        op1=ALU.add, accum_out=Tt[:])
                cnt = smallp.tile([128, 1], F32, tag="bcnt")
                nc.scalar.activation(cjunk[:], gata[:], AF.Sign,
                                     scale=-1.0, bias=Tt[:], accum_out=cnt[:])
                cntb = smallp.tile([128, 1], BF16, tag="bcntb")
                nc.vector.tensor_copy(cntb[:], cnt[:])
                psum_c = psb.tile([128, 1], F32, tag="bps")
                nc.tensor.matmul(psum_c[:], onesKM[:], cntb[:],
                                 start=True, stop=True)
                m_le = smallp.tile([128, 1], I32, tag="bmle")
                m_gt = smallp.tile([128, 1], I32, tag="bmgt")
                nc.vector.tensor_scalar(out=m_le[:], in0=psum_c[:],
                                        scalar1=D0s, scalar2=None,
                                        op0=ALU.is_le)
                nc.vector.tensor_scalar(out=m_gt[:], in0=psum_c[:],
                                        scalar1=D0s, scalar2=None,
                                        op0=ALU.is_gt)
                nc.vector.copy_predicated(out=lh[:, 0:1], mask=m_le[:],
                                          data=Tt[:])
                nc.vector.copy_predicated(out=lh[:, 1:2], mask=m_gt[:],
                                          data=Tt[:])

            def ps_a_view(ps):
                # A-half [64, t, b, 8, 0:8] view of grouped psum (full groups)
                return ps[0:64, :].rearrange(
                    "p (t x) -> p t x", t=GROUP_T, x=512)[:, :, 0:432] \
                    .rearrange("p t (b r w) -> p t b r w",
                               b=TILE_B, r=8, w=9)[:, :, :, :, 0:8]

            def ps_b_view(ps):
                return ps[64:128, :].rearrange(
                    "p (t x) -> p t x", t=GROUP_T, x=512)[:, :, 0:432] \
                    .rearrange("p t (b r w) -> p t b r w",
                               b=TILE_B, r=8, w=9)[:, :, :, :, 1:9]

            def conv_group(gi, nbs, lhs, pad):
                """3*GROUP_T matmuls (dy-major); B-half bounced to parts 0:64."""
                slot = gi % NSLOT
                ful = all(nb == TILE_B for nb in nbs)
                ps = psc.tile([128, GROUP_T * 512], F32, tag="cps")
                for dy in range(3):
                    for (t, nb) in enumerate(nbs):
                        nc.tensor.matmul(
                            ps[:, 512 * t:512 * t + nb * 72].rearrange(
                                "p (b r w) -> p b r w", b=nb, r=8, w=9),
                            lhs[dy][:],
                            pad[:, slot, TILE_B * t:TILE_B * t + nb,
                                dy:dy + 8, :],
                            start=(dy == 0), stop=(dy == 2))
                cmb = prep.tile([128, GROUP_T, TILE_B, 8, 8], BF16, tag="cmb")
                for (t, nb) in enumerate(nbs):
                    src = ps[64:128, 512 * t:512 * t + nb * 72].rearrange(
                        "p (b r w) -> p b r w", b=nb, r=8, w=9)[:, :, :, 1:9]
                    if (gi + t) % 2 == 0:
                        nc.scalar.copy(cmb[64:128, t, 0:nb], src)
                    else:
                        nc.vector.tensor_copy(cmb[64:128, t, 0:nb], src)
                if ful:
                    nc.sync.dma_start(cmb[0:64], cmb[64:128])
                else:
                    for (t, nb) in enumerate(nbs):
                        nc.sync.dma_start(cmb[0:64, t, 0:nb],
                                          cmb[64:128, t, 0:nb])
                return ps, cmb

            # ---------------- conv1 + interleaved bisection ----------------
            for (gi, (b0, nbs)) in enumerate(groups):
                slot = gi % NSLOT
                ns = sum(nbs)
                ful = all(nb == TILE_B for nb in nbs)
                stg = stgp.tile([C, GB, 8, 8], F32, tag="stg")
                nc.sync.dma_start(stg[:, 0:ns], x_src(b0, ns))
                if ful:
                    nc.scalar.activation(
                        xpad[0:64, slot, :, 1:9, 1:9], stg[:], AF.Copy)
                else:
                    st = 0
                    for (t, nb) in enumerate(nbs):
                        nc.scalar.activation(
                            xpad[0:64, slot, TILE_B * t:TILE_B * t + nb,
                                 1:9, 1:9],
                            stg[:, st:st + nb], AF.Copy)
                        st += nb
                # flat shift-by-one duplicate (single contiguous run / part)
                nc.sync.dma_start(xpad_f[64:128, slot, 0:GB * PADSZ - 1],
                                  xpad_f[0:64, slot, 1:GB * PADSZ])
                ps, cmb = conv_group(gi, nbs, lhs1, xpad)
                for (t, nb) in enumerate(nbs):
                    nc.vector.tensor_tensor(
                        out=R[:, GROUP_T * gi + t, 0:nb].rearrange(
                            "p b (h w) -> p b h w", h=8, w=8),
                        in0=ps[0:64, 512 * t:512 * t + nb * 72].rearrange(
                            "p (b r w) -> p b r w", b=nb, r=8, w=9)
                        [:, :, :, 0:8],
                        in1=cmb[0:64, t, 0:nb], op=ALU.add)

                for _ in range(bis_at.get(gi, 0)):
                    bisect_iter()

            # final threshold -> -T
            tj = smallp.tile([128, 2], F32, tag="bj")
            nc.vector.tensor_scalar(out=tj[:], in0=lh[:], scalar1=0.5,
                                    scalar2=None, op0=ALU.mult,
                                    op1=ALU.add, accum_out=Tt[:])
            negT = pp.tile([128, 1], F32, tag="negT")
            nc.vector.tensor_scalar(out=negT[:], in0=Tt[:], scalar1=-1.0,
                                    scalar2=None, op0=ALU.mult)
            gatap_cm.__exit__(None, None, None)

            # ---------------- P3a: gate application + BN1 stats ----------------
            nc.scalar.activation(gates[:], gates[:], AF.Relu,
                                 bias=negT[0:64, :])
            sep = gates
            for (gi, (b0, nbs)) in enumerate(groups):
                ns = sum(nbs)
                if all(nb == TILE_B for nb in nbs):
                    rsl = R[:, GROUP_T * gi:GROUP_T * (gi + 1)].rearrange(
                        "p t b q -> p (t b) q")
                    sep_b = sep[:, b0:b0 + ns].unsqueeze(2).broadcast_to(
                        (C, ns, HW))
                    ti0 = GROUP_T * gi
                    nc.vector.scalar_tensor_tensor(
                        out=rsl, in0=rsl, scalar=1.0, in1=sep_b,
                        op0=ALU.mult, op1=ALU.mult,
                        accum_out=stats[:, ti0:ti0 + 1])
                    sqj = prep.tile([C, GB, HW], F32, tag="pre")
                    nc.scalar.activation(
                        sqj[:].rearrange("p b q -> p (b q)"),
                        rsl.rearrange("p b q -> p (b q)"), AF.Square,
                        accum_out=stats[:, NT + ti0:NT + ti0 + 1])
                else:
                    st = 0
                    for (t, nb) in enumerate(nbs):
                        ti = GROUP_T * gi + t
                        rsl = R[:, ti, 0:nb]
                        sep_b = sep[:, b0 + st:b0 + st + nb].unsqueeze(
                            2).broadcast_to((C, nb, HW))
                        nc.vector.scalar_tensor_tensor(
                            out=rsl, in0=rsl, scalar=1.0, in1=sep_b,
                            op0=ALU.mult, op1=ALU.mult,
                            accum_out=stats[:, ti:ti + 1])
                        sqj = prep.tile([C, GB, HW], F32, tag="pre")
                        nc.scalar.activation(
                            sqj[:, 0:nb].rearrange("p b q -> p (b q)"),
                            rsl.rearrange("p b q -> p (b q)"), AF.Square,
                            accum_out=stats[:, NT + ti:NT + ti + 1])
                        st += nb

            def stats_allreduce(c0, arin, arout, cf, gcol, bcol):
                nc.vector.tensor_reduce(
                    out=sq_l[:, 0:1], in_=stats[:, c0 * NT:(c0 + 1) * NT],
                    axis=AX.X, op=ALU.add)
                nc.vector.tensor_reduce(
                    out=sq_l[:, 1:2],
                    in_=stats[:, (c0 + 1) * NT:(c0 + 2) * NT],
                    axis=AX.X, op=ALU.add)
                nc.sync.dma_start(arin[:], sq_l[:, 0:2])
                nc.gpsimd.collective_compute(
                    "AllReduce", ALU.add, replica_groups=rg,
                    ins=[arin.opt()], outs=[arout.opt()])
                sq_g = smallp.tile([C, 2], F32, tag="sqg")
                nc.sync.dma_start(sq_g[:], arout[:])
                # scratch cols: 0=mean 1=E[x^2] 2=-var 3=sd 4=isd
                nc.vector.tensor_scalar(out=scratch[:, 0:2], in0=sq_g[:],
                                        scalar1=1.0 / N1, scalar2=None,
                                        op0=ALU.mult)
                nc.vector.scalar_tensor_tensor(
                    out=scratch[:, 2:3], in0=scratch[:, 0:1],
                    scalar=scratch[:, 0:1], in1=scratch[:, 1:2],
                    op0=ALU.mult, op1=ALU.subtract)
                nc.scalar.activation(scratch[:, 3:4], scratch[:, 2:3],
                                     AF.Sqrt, scale=-1.0, bias=eps_t[:])
                nc.vector.reciprocal(scratch[:, 4:5], scratch[:, 3:4])
                nc.vector.tensor_tensor(out=cf[:, 0:1],
                                        in0=vecs[:, gcol:gcol + 1],
                                        in1=scratch[:, 4:5], op=ALU.mult)
                nc.vector.scalar_tensor_tensor(
                    out=cf[:, 1:2], in0=scratch[:, 0:1],
                    scalar=cf[:, 0:1], in1=vecs[:, bcol:bcol + 1],
                    op0=ALU.mult, op1=ALU.subtract)
                nc.vector.tensor_scalar(out=cf[:, 1:2], in0=cf[:, 1:2],
                                        scalar1=-1.0, scalar2=None,
                                        op0=ALU.mult)

            stats_allreduce(0, ar_in, ar_out, cf1, 1, 2)

            # ------------- P3b: bn1+relu -> conv2 -> BN2 stats -------------
            for (gi, (b0, nbs)) in enumerate(groups):
                slot = gi % NSLOT
                ful = all(nb == TILE_B for nb in nbs)
                if ful:
                    nc.scalar.activation(
                        ypad[0:64, slot, :, 1:9, 1:9],
                        R[:, GROUP_T * gi:GROUP_T * (gi + 1)].rearrange(
                            "p t b (h w) -> p (t b) h w", h=8, w=8),
                        AF.Relu, scale=cf1[:, 0:1], bias=cf1[:, 1:2])
                else:
                    for (t, nb) in enumerate(nbs):
                        nc.scalar.activation(
                            ypad[0:64, slot, TILE_B * t:TILE_B * t + nb,
                                 1:9, 1:9],
                            R[:, GROUP_T * gi + t, 0:nb].rearrange(
                                "p b (h w) -> p b h w", h=8, w=8),
                            AF.Relu, scale=cf1[:, 0:1], bias=cf1[:, 1:2])
                nc.sync.dma_start(ypad_f[64:128, slot, 0:GB * PADSZ - 1],
                                  ypad_f[0:64, slot, 1:GB * PADSZ])
                ps, cmb = conv_group(gi, nbs, lhs2, ypad)
                for (t, nb) in enumerate(nbs):
                    ti = GROUP_T * gi + t
                    rsl = R[:, ti, 0:nb]
                    nc.vector.scalar_tensor_tensor(
                        out=rsl.rearrange("p b (h w) -> p b h w", h=8, w=8),
                        in0=ps[0:64, 512 * t:512 * t + nb * 72].rearrange(
                            "p (b r w) -> p b r w", b=nb, r=8, w=9)
                        [:, :, :, 0:8],
                        scalar=1.0, in1=cmb[0:64, t, 0:nb],
                        op0=ALU.mult, op1=ALU.add,
                        accum_out=stats[:, 2 * NT + ti:2 * NT + ti + 1])
                if ful:
                    ti0 = GROUP_T * gi
                    sqj = prep.tile([C, GB, HW], F32, tag="pre")
                    rfl = R[:, GROUP_T * gi:GROUP_T * (gi + 1)].rearrange(
                        "p t b q -> p (t b q)")
                    if gi % 2 == 1:
                        nc.scalar.activation(
                            sqj[:].rearrange("p b q -> p (b q)"), rfl,
                            AF.Square,
                            accum_out=stats[:, 3 * NT + ti0:3 * NT + ti0 + 1])
                    else:
                        nc.vector.scalar_tensor_tensor(
                            out=sqj[:].rearrange("p b q -> p (b q)"), in0=rfl,
                            scalar=1.0, in1=rfl, op0=ALU.mult, op1=ALU.mult,
                            accum_out=stats[:, 3 * NT + ti0:3 * NT + ti0 + 1])
                else:
                    for (t, nb) in enumerate(nbs):
                        ti = GROUP_T * gi + t
                        rsl = R[:, ti, 0:nb]
                        sqj = prep.tile([C, GB, HW], F32, tag="pre")
                        nc.scalar.activation(
                            sqj[:, 0:nb].rearrange("p b q -> p (b q)"),
                            rsl.rearrange("p b q -> p (b q)"), AF.Square,
                            accum_out=stats[:, 3 * NT + ti:3 * NT + ti + 1])

            stats_allreduce(2, ar2_in, ar2_out, cf2, 3, 4)

            # ---------------- P5: bn2 + residual + relu -> out ----------------
            latep_cm = tc.tile_pool(name="latep", bufs=3)
            latep = latep_cm.__enter__()
            for (gi, (b0, nbs)) in enumerate(groups):
                ns = sum(nbs)
                ful = all(nb == TILE_B for nb in nbs)
                stg = latep.tile([C, GB, 8, 8], F32, tag="stg5")
                nc.sync.dma_start(stg[:, 0:ns], x_src(b0, ns))
                pre = prep.tile([C, GB, HW], F32, tag="pre")
                if ful:
                    rsl = R[:, GROUP_T * gi:GROUP_T * (gi + 1)].rearrange(
                        "p t b q -> p (t b) q")
                    nc.vector.scalar_tensor_tensor(
                        out=pre[:], in0=rsl, scalar=cf2[:, 0:1],
                        in1=stg[:].rearrange("p b h w -> p b (h w)"),
                        op0=ALU.mult, op1=ALU.add)
                else:
                    st = 0
                    for (t, nb) in enumerate(nbs):
                        rsl = R[:, GROUP_T * gi + t, 0:nb]
                        nc.vector.scalar_tensor_tensor(
                            out=pre[:, st:st + nb], in0=rsl,
                            scalar=cf2[:, 0:1],
                            in1=stg[:, st:st + nb].rearrange(
                                "p b h w -> p b (h w)"),
                            op0=ALU.mult, op1=ALU.add)
                        st += nb
                nc.scalar.activation(pre[:, 0:ns], pre[:, 0:ns], AF.Relu,
                                     bias=cf2[:, 1:2])
                nc.sync.dma_start(
                    out_d[b0:b0 + ns].transpose([1, 0, 2, 3]),
                    pre[:, 0:ns].rearrange("p b (h w) -> p b h w", h=8, w=8))

            latep_cm.__exit__(None, None, None)
            psb_cm.__exit__(None, None, None)
            psc_cm.__exit__(None, None, None)

    nc.compile()
    return nc


_NC_CACHE = {}


def _get_nc(n_cores, b_loc):
    key = (n_cores, b_loc)
    if key not in _NC_CACHE:
        _NC_CACHE[key] = build_nc(n_cores, b_loc)
    return _NC_CACHE[key]


def kernel(**inputs):
    from concourse.bass_utils import run_bass_kernel_spmd

    x = np.asarray(inputs["x"], dtype=np.float32)
    B = x.shape[0]
    n_cores = 8
    b_loc = B // n_cores
    nc = _get_nc(n_cores, b_loc)

    weight_names = ["conv1_w", "conv2_w", "fc1_w", "fc1_b", "fc2_w", "fc2_b",
                    "bn1_g", "bn1_b", "bn2_g", "bn2_b"]
    in_maps = []
    for c in range(n_cores):
        m = {"x": np.ascontiguousarray(x[c * b_loc:(c + 1) * b_loc])}
        for n in weight_names:
            m[n] = np.asarray(inputs[n], dtype=np.float32)
        in_maps.append(m)
    res = run_bass_kernel_spmd(nc, in_maps, core_ids=list(range(n_cores)))
    out = np.concatenate([res.results[c]["out"] for c in range(n_cores)],
                         axis=0)
    return out.astype(np.float32)


# revision 24
# speedup vs baseline: 1.2338x; 1.0175x over previous
"""Trainium2 Bass kernel for nn_BasicBlock (conv-SE-prune-BN residual block).

Data-parallel over batch across 8 NeuronCores. Per core (B_loc = 1024),
processed in groups of 3 six-sample conv tiles (18 samples):
  P0  : stream x, per-(channel,sample) spatial mean (pooling)
  fc  : fc1-relu-fc2-sigmoid gates (tiny PE matmuls)
  AG  : AllGather all B*C gates; global-threshold bisection (22 fixed
        count-below-T iterations on ACT, interleaved with conv1)
  conv1: 3x3 conv = 3 K=128/M=128 bf16 matmuls per tile; K halves are
        channels + a flat-shifted duplicate (one contiguous SBUF DMA);
        M halves are two accumulators, combined via an SBUF bounce
  P3a : out1 * relu(gate - T), BN1 partial stats
  AR2 : AllReduce BN1 stats -> affine coefs
  P3b : bn1-affine+relu -> conv2 -> BN2 partial stats
  AR3 : AllReduce BN2 stats
  P5  : bn2-affine + residual + relu -> out

kernel(**inputs) takes the FULL inputs and returns the FULL output.
"""
import numpy as np

import concourse.bacc as bacc
import concourse.bass as bass
import concourse.mybir as mybir
import concourse.tile as tile

F32 = mybir.dt.float32
BF16 = mybir.dt.bfloat16
I32 = mybir.dt.int32
AF = mybir.ActivationFunctionType
ALU = mybir.AluOpType
AX = mybir.AxisListType

C = 64
HW = 64          # 8*8 spatial
TILE_B = 6
GROUP_T = 3      # conv tiles per group (shared psum tensor / DMAs)
PRUNE_RATE = 0.2
EPS = 1e-5
BISECT_ITERS = 22
PADSZ = 10 * 9   # padded sample size


def _groups(b_loc):
    """[(b0, [nb per tile])]; all but possibly the last have full tiles."""
    tiles = []
    b0 = 0
    while b0 < b_loc:
        nb = min(TILE_B, b_loc - b0)
        tiles.append((b0, nb))
        b0 += nb
    out = []
    i = 0
    while i < len(tiles):
        grp = tiles[i:i + GROUP_T]
        out.append((grp[0][0], [nb for (_, nb) in grp]))
        i += GROUP_T
    return out


def _transpose64(nc, dst_ap, src_ap):
    # full 64x64 transpose from per-32-block vector.transpose
    for i in (0, 32):
        for j in (0, 32):
            nc.vector.transpose(out=dst_ap[j:j + 32, i:i + 32],
                                in_=src_ap[i:i + 32, j:j + 32])


def build_nc(n_cores, b_loc):
    B_glob = n_cores * b_loc
    k_prune = int(PRUNE_RATE * B_glob * C)
    G = (b_loc * C * n_cores) // 128
    # sum of sign(T-g) = 2*count_less - N ; count_less <= k <=> sum <= 2k-N
    D0s = float(2 * k_prune - B_glob * C)
    N1 = float(B_glob * HW)
    groups = _groups(b_loc)
    NG = len(groups)
    rg = [list(range(n_cores))]
    GB = GROUP_T * TILE_B

    nc = bacc.Bacc("TRN2", target_bir_lowering=False, debug=False,
                   enable_asserts=True, num_devices=n_cores)

    x_in = nc.dram_tensor("x", [b_loc, C, 8, 8], F32, kind="ExternalInput")
    w1_in = nc.dram_tensor("conv1_w", [C, C, 3, 3], F32, kind="ExternalInput")
    w2_in = nc.dram_tensor("conv2_w", [C, C, 3, 3], F32, kind="ExternalInput")
    fc1w_in = nc.dram_tensor("fc1_w", [16, C], F32, kind="ExternalInput")
    fc1b_in = nc.dram_tensor("fc1_b", [16], F32, kind="ExternalInput")
    fc2w_in = nc.dram_tensor("fc2_w", [C, 16], F32, kind="ExternalInput")
    fc2b_in = nc.dram_tensor("fc2_b", [C], F32, kind="ExternalInput")
    bn1g_in = nc.dram_tensor("bn1_g", [C], F32, kind="ExternalInput")
    bn1b_in = nc.dram_tensor("bn1_b", [C], F32, kind="ExternalInput")
    bn2g_in = nc.dram_tensor("bn2_g", [C], F32, kind="ExternalInput")
    bn2b_in = nc.dram_tensor("bn2_b", [C], F32, kind="ExternalInput")
    out_d = nc.dram_tensor("out", [b_loc, C, 8, 8], F32, kind="ExternalOutput")

    with tile.TileContext(nc) as tc:
        with (
            tc.tile_pool(name="persist", bufs=1) as pp,
            tc.tile_pool(name="stg", bufs=2) as stgp,
            tc.tile_pool(name="pads", bufs=1) as padp,
            tc.tile_pool(name="small", bufs=2) as smallp,
            tc.tile_pool(name="prer", bufs=2) as prep,
            tc.tile_pool(name="dram", bufs=1, space="DRAM") as dramp,
        ):
            # early dummy collective absorbs cross-core start skew
            bar_sb = pp.tile([1, 1], F32, tag="bar_sb")
            bar_in = dramp.tile([1, 1], F32, tag="bar_in")
            bar_out = dramp.tile([1, 1], F32, tag="bar_out")
            nc.vector.memset(bar_sb[:], 0)
            nc.sync.dma_start(bar_in[:], bar_sb[:])
            nc.gpsimd.collective_compute(
                "AllReduce", ALU.add, replica_groups=rg,
                ins=[bar_in.opt()], outs=[bar_out.opt()])

            # ---------------- constants / weights prep ----------------
            w1_sb = pp.tile([C, C, 3, 3], F32, tag="w1")
            w2_sb = pp.tile([C, C, 3, 3], F32, tag="w2")
            nc.sync.dma_start(w1_sb[:], w1_in[:])
            nc.sync.dma_start(w2_sb[:], w2_in[:])
            lhs1, lhs2 = [], []
            for (wsb, lst, nm) in ((w1_sb, lhs1, "l1"), (w2_sb, lhs2, "l2")):
                for dy in range(3):
                    lt = pp.tile([128, 128], BF16, tag=f"{nm}_{dy}")
                    nc.vector.memset(lt[:], 0)
                    for (kp, mp, dx) in ((0, 0, 0), (64, 0, 1), (64, 64, 2)):
                        tp = smallp.tile([C, C], F32, tag="wtr")
                        _transpose64(nc, tp[:], wsb[:, :, dy, dx])
                        nc.vector.tensor_copy(lt[kp:kp + 64, mp:mp + 64], tp[:])
                    lst.append(lt)

            fc1T = pp.tile([C, C], F32, tag="fc1T")   # [64, 16] used
            fc2T = pp.tile([C, C], F32, tag="fc2T")   # [16, 64] used
            for (w_in_, shape, dstT) in ((fc1w_in, (16, C), fc1T),
                                         (fc2w_in, (C, 16), fc2T)):
                tmp = smallp.tile([C, C], F32, tag="fctmp")
                nc.vector.memset(tmp[:], 0)
                nc.sync.dma_start(tmp[0:shape[0], 0:shape[1]], w_in_[:])
                _transpose64(nc, dstT[:], tmp[:])

            vecs = pp.tile([C, 8], F32, tag="vecs")
            # cols: 0=fc2_b 1=bn1_g 2=bn1_b 3=bn2_g 4=bn2_b
            nc.sync.dma_start(vecs[:, 0:1], fc2b_in[:].unsqueeze(1))
            nc.sync.dma_start(vecs[:, 1:2], bn1g_in[:].unsqueeze(1))
            nc.sync.dma_start(vecs[:, 2:3], bn1b_in[:].unsqueeze(1))
            nc.sync.dma_start(vecs[:, 3:4], bn2g_in[:].unsqueeze(1))
            nc.sync.dma_start(vecs[:, 4:5], bn2b_in[:].unsqueeze(1))
            fc1b = pp.tile([16, 1], F32, tag="fc1b")
            nc.sync.dma_start(fc1b[:], fc1b_in[:].unsqueeze(1))

            onesKM = pp.tile([128, 128], BF16, tag="ones")
            nc.vector.memset(onesKM[:], 1.0)
            eps_t = pp.tile([C, 1], F32, tag="eps")
            nc.vector.memset(eps_t[:], EPS)

            # padded-input ring [128, slot, GB, 10, 9]; borders stay 0.
            NSLOT = 2
            xpad = padp.tile([128, NSLOT, GB, 10, 9], BF16, tag="xpad")
            nc.vector.memset(xpad[:], 0)
            xpad_f = xpad[:].rearrange("p s b r w -> p s (b r w)")

            NT = sum(len(nbs) for (_, nbs) in groups)
            R = pp.tile([C, NT, TILE_B, HW], BF16, tag="R")
            pooled = pp.tile([C, b_loc], F32, tag="pooled")
            gates = pp.tile([C, b_loc], F32, tag="gates")
            # stats sections of NT per-tile cols: S1, Q1, S2, Q2 (merged
            # group ops write their sum into the group's first tile column)
            stats = pp.tile([C, 4 * NT], F32, tag="stats")
            nc.vector.memset(stats[:], 0)
            sq_l = pp.tile([C, 4], F32, tag="sq_l")
            cf1 = pp.tile([C, 2], F32, tag="cf1")
            cf2 = pp.tile([C, 2], F32, tag="cf2")
            scratch = pp.tile([C, 8], F32, tag="scratch")

            # dram bounce buffers for collectives
            ag_in = dramp.tile([C, b_loc], F32, tag="ag_in")
            ag_out = dramp.tile([n_cores, C, b_loc], F32, tag="ag_out")
            ar_in = dramp.tile([C, 2], F32, tag="ar_in")
            ar_out = dramp.tile([C, 2], F32, tag="ar_out")
            ar2_in = dramp.tile([C, 2], F32, tag="ar2_in")
            ar2_out = dramp.tile([C, 2], F32, tag="ar2_out")

            def x_src(b0, ns):
                return x_in[b0:b0 + ns].transpose([1, 0, 2, 3])

            # ---------------- P0: pooling pass ----------------
            # bigger chunks than the conv groups: the gates buffers are not
            # allocated yet, so borrow that SBUF for 36-sample staging
            p0_cm = tc.tile_pool(name="p0stg", bufs=2)
            p0p = p0_cm.__enter__()
            P0C = 2 * GB
            b0 = 0
            while b0 < b_loc:
                ns = min(P0C, b_loc - b0)
                stg = p0p.tile([C, P0C, 8, 8], F32, tag="stg0")
                nc.sync.dma_start(stg[:, 0:ns], x_src(b0, ns))
                nc.vector.tensor_reduce(out=pooled[:, b0:b0 + ns],
                                        in_=stg[:, 0:ns], axis=AX.XY,
                                        op=ALU.add)
                b0 += ns
            p0_cm.__exit__(None, None, None)

            gatap_cm = tc.tile_pool(name="gatap", bufs=1)
            gatap = gatap_cm.__enter__()
            gata = gatap.tile([128, G], F32, tag="gata")
            cjunk = gatap.tile([128, G], BF16, tag="cjunk")

            # ---------- gates: fc1 relu fc2 sigmoid (scoped psum) ----------
            with tc.tile_pool(name="ps_fc", bufs=2, space="PSUM") as psm:
                # z1 is overlaid on pooled[0:16] (each chunk read before write)
                for j in range(0, b_loc, 512):
                    e = min(j + 512, b_loc)
                    zp = psm.tile([C, 512], F32, tag="zfc")
                    nc.tensor.matmul(zp[0:16, 0:e - j], fc1T[:, 0:16],
                                     pooled[:, j:e], start=True, stop=True)
                    nc.scalar.activation(pooled[0:16, j:e], zp[0:16, 0:e - j],
                                         AF.Relu, scale=1.0 / HW, bias=fc1b[:])
                for j in range(0, b_loc, 512):
                    e = min(j + 512, b_loc)
                    zp = psm.tile([C, 512], F32, tag="zfc")
                    nc.tensor.matmul(zp[:, 0:e - j], fc2T[0:16, :],
                                     pooled[0:16, j:e], start=True, stop=True)
                    nc.scalar.activation(gates[:, j:e], zp[:, 0:e - j],
                                         AF.Sigmoid, bias=vecs[:, 0:1])

            # allgather gates, load as [128, G]
            nc.sync.dma_start(ag_in[:], gates[:])
            nc.gpsimd.collective_compute(
                "AllGather", ALU.bypass, replica_groups=rg,
                ins=[ag_in.opt()], outs=[ag_out.opt()])
            nc.sync.dma_start(
                gata[:], ag_out[:].rearrange("n c b -> (n c b)")
                .rearrange("(p g) -> p g", p=128))

            psc_cm = tc.tile_pool(name="ps_conv", bufs=2, space="PSUM")
            psc = psc_cm.__enter__()
            psb_cm = tc.tile_pool(name="ps_bis", bufs=2, space="PSUM")
            psb = psb_cm.__enter__()

            # ---------------- bisection machinery ----------------
            lh = pp.tile([128, 2], F32, tag="lh")
            Tt = pp.tile([128, 1], F32, tag="Tt")
            nc.vector.memset(lh[:, 0:1], 0.0)
            nc.vector.memset(lh[:, 1:2], 1.0)

            bis_at = {}
            bstart = NG - 2 - 2 * (BISECT_ITERS - 1)
            if bstart >= 1:
                for j in range(BISECT_ITERS):
                    bis_at[bstart + 2 * j] = 1
            else:
                bis_at[max(0, NG - 2)] = BISECT_ITERS

            def bisect_iter():
                tj = smallp.tile([128, 2], F32, tag="bj")
                nc.vector.tensor_scalar(out=tj[:], in0=lh[:], scalar1=0.5,
                                        scalar2=None, op0=ALU.mult,
                                        op1=ALU.add, accum_out=Tt[:])
                cnt = smallp.tile([128, 1], F32, tag="bcnt")
                nc.scalar.activation(cjunk[:], gata[:], AF.Sign,
                                     scale=-1.0, bias=Tt[:], accum_out=cnt[:])
                cntb = smallp.tile([128, 1], BF16, tag="bcntb")
                nc.vector.tensor_copy(cntb[:], cnt[:])
                psum_c = psb.tile([128, 1], F32, tag="bps")
                nc.tensor.matmul(psum_c[:], onesKM[:], cntb[:],
                                 start=True, stop=True)
                m_le = smallp.tile([128, 1], I32, tag="bmle")
                m_gt = smallp.tile([128, 1], I32, tag="bmgt")
                nc.vector.tensor_scalar(out=m_le[:], in0=psum_c[:],
                                        scalar1=D0s, scalar2=None,
                                        op0=ALU.is_le)
                nc.vector.tensor_scalar(out=m_gt[:], in0=psum_c[:],
                                        scalar1=D0s, scalar2=None,
                                        op0=ALU.is_gt)
                nc.vector.copy_predicated(out=lh[:, 0:1], mask=m_le[:],
                                          data=Tt[:])
                nc.vector.copy_predicated(out=lh[:, 1:2], mask=m_gt[:],
                                          data=Tt[:])

            def ps_a_view(ps):
                # A-half [64, t, b, 8, 0:8] view of grouped psum (full groups)
                return ps[0:64, :].rearrange(
                    "p (t x) -> p t x", t=GROUP_T, x=512)[:, :, 0:432] \
                    .rearrange("p t (b r w) -> p t b r w",
                               b=TILE_B, r=8, w=9)[:, :, :, :, 0:8]

            def ps_b_view(ps):
                return ps[64:128, :].rearrange(
                    "p (t x) -> p t x", t=GROUP_T, x=512)[:, :, 0:432] \
                    .rearrange("p t (b r w) -> p t b r w",
                               b=TILE_B, r=8, w=9)[:, :, :, :, 1:9]

            def conv_group(gi, slot, nbs, lhs, pad):
                """3*GROUP_T matmuls (dy-major); B-half bounced to parts 0:64."""
                ful = all(nb == TILE_B for nb in nbs)
                ps = psc.tile([128, GROUP_T * 512], F32, tag="cps")
                for dy in range(3):
                    for (t, nb) in enumerate(nbs):
                        nc.tensor.matmul(
                            ps[:, 512 * t:512 * t + nb * 72].rearrange(
                                "p (b r w) -> p b r w", b=nb, r=8, w=9),
                            lhs[dy][:],
                            pad[:, slot, TILE_B * t:TILE_B * t + nb,
                                dy:dy + 8, :],
                            start=(dy == 0), stop=(dy == 2))
                cmb = prep.tile([128, GROUP_T, TILE_B, 8, 8], BF16, tag="cmb")
                for (t, nb) in enumerate(nbs):
                    src = ps[64:128, 512 * t:512 * t + nb * 72].rearrange(
                        "p (b r w) -> p b r w", b=nb, r=8, w=9)[:, :, :, 1:9]
                    if (gi + t) % 2 == 0:
                        nc.scalar.copy(cmb[64:128, t, 0:nb], src)
                    else:
                        nc.vector.tensor_copy(cmb[64:128, t, 0:nb], src)
                if ful:
                    nc.sync.dma_start(cmb[0:64], cmb[64:128])
                else:
                    for (t, nb) in enumerate(nbs):
                        nc.sync.dma_start(cmb[0:64, t, 0:nb],
                                          cmb[64:128, t, 0:nb])
                return ps, cmb

            # ---------------- conv1 + interleaved bisection ----------------
            for (gi, (b0, nbs)) in enumerate(groups):
                slot = gi % NSLOT
                ns = sum(nbs)
                ful = all(nb == TILE_B for nb in nbs)
                stg = stgp.tile([C, GB, 8, 8], F32, tag="stg")
                nc.sync.dma_start(stg[:, 0:ns], x_src(b0, ns))
                if ful:
                    nc.scalar.activation(
                        xpad[0:64, slot, :, 1:9, 1:9], stg[:], AF.Copy)
                else:
                    st = 0
                    for (t, nb) in enumerate(nbs):
                        nc.scalar.activation(
                            xpad[0:64, slot, TILE_B * t:TILE_B * t + nb,
                                 1:9, 1:9],
                            stg[:, st:st + nb], AF.Copy)
                        st += nb
                # flat shift-by-one duplicate (single contiguous run / part)
                nc.sync.dma_start(xpad_f[64:128, slot, 0:GB * PADSZ - 1],
                                  xpad_f[0:64, slot, 1:GB * PADSZ])
                ps, cmb = conv_group(gi, slot, nbs, lhs1, xpad)
                for (t, nb) in enumerate(nbs):
                    nc.vector.tensor_tensor(
                        out=R[:, GROUP_T * gi + t, 0:nb].rearrange(
                            "p b (h w) -> p b h w", h=8, w=8),
                        in0=ps[0:64, 512 * t:512 * t + nb * 72].rearrange(
                            "p (b r w) -> p b r w", b=nb, r=8, w=9)
                        [:, :, :, 0:8],
                        in1=cmb[0:64, t, 0:nb], op=ALU.add)

                for _ in range(bis_at.get(gi, 0)):
                    bisect_iter()

            # final threshold -> -T
            tj = smallp.tile([128, 2], F32, tag="bj")
            nc.vector.tensor_scalar(out=tj[:], in0=lh[:], scalar1=0.5,
                                    scalar2=None, op0=ALU.mult,
                                    op1=ALU.add, accum_out=Tt[:])
            negT = pp.tile([128, 1], F32, tag="negT")
            nc.vector.tensor_scalar(out=negT[:], in0=Tt[:], scalar1=-1.0,
                                    scalar2=None, op0=ALU.mult)
            gatap_cm.__exit__(None, None, None)

            # ---------------- P3a: gate application + BN1 stats ----------------
            nc.scalar.activation(gates[:], gates[:], AF.Relu,
                                 bias=negT[0:64, :])
            sep = gates
            for (gi, (b0, nbs)) in enumerate(groups):
                ns = sum(nbs)
                if all(nb == TILE_B for nb in nbs):
                    rsl = R[:, GROUP_T * gi:GROUP_T * (gi + 1)].rearrange(
                        "p t b q -> p (t b) q")
                    sep_b = sep[:, b0:b0 + ns].unsqueeze(2).broadcast_to(
                        (C, ns, HW))
                    ti0 = GROUP_T * gi
                    nc.vector.scalar_tensor_tensor(
                        out=rsl, in0=rsl, scalar=1.0, in1=sep_b,
                        op0=ALU.mult, op1=ALU.mult,
                        accum_out=stats[:, ti0:ti0 + 1])
                    sqj = prep.tile([C, GB, HW], F32, tag="pre")
                    nc.scalar.activation(
                        sqj[:].rearrange("p b q -> p (b q)"),
                        rsl.rearrange("p b q -> p (b q)"), AF.Square,
                        accum_out=stats[:, NT + ti0:NT + ti0 + 1])
                else:
                    st = 0
                    for (t, nb) in enumerate(nbs):
                        ti = GROUP_T * gi + t
                        rsl = R[:, ti, 0:nb]
                        sep_b = sep[:, b0 + st:b0 + st + nb].unsqueeze(
                            2).broadcast_to((C, nb, HW))
                        nc.vector.scalar_tensor_tensor(
                            out=rsl, in0=rsl, scalar=1.0, in1=sep_b,
                            op0=ALU.mult, op1=ALU.mult,
                            accum_out=stats[:, ti:ti + 1])
                        sqj = prep.tile([C, GB, HW], F32, tag="pre")
                        nc.scalar.activation(
                            sqj[:, 0:nb].rearrange("p b q -> p (b q)"),
                            rsl.rearrange("p b q -> p (b q)"), AF.Square,
                            accum_out=stats[:, NT + ti:NT + ti + 1])
                        st += nb

            def stats_allreduce(c0, arin, arout, cf, gcol, bcol):
                nc.vector.tensor_reduce(
                    out=sq_l[:, 0:1], in_=stats[:, c0 * NT:(c0 + 1) * NT],
                    axis=AX.X, op=ALU.add)
                nc.vector.tensor_reduce(
                    out=sq_l[:, 1:2],
                    in_=stats[:, (c0 + 1) * NT:(c0 + 2) * NT],
                    axis=AX.X, op=ALU.add)
                nc.sync.dma_start(arin[:], sq_l[:, 0:2])
                nc.gpsimd.collective_compute(
                    "AllReduce", ALU.add, replica_groups=rg,
                    ins=[arin.opt()], outs=[arout.opt()])
                sq_g = smallp.tile([C, 2], F32, tag="sqg")
                nc.sync.dma_start(sq_g[:], arout[:])
                # scratch cols: 0=mean 1=E[x^2] 2=-var 3=sd 4=isd
                nc.vector.tensor_scalar(out=scratch[:, 0:2], in0=sq_g[:],
                                        scalar1=1.0 / N1, scalar2=None,
                                        op0=ALU.mult)
                nc.vector.scalar_tensor_tensor(
                    out=scratch[:, 2:3], in0=scratch[:, 0:1],
                    scalar=scratch[:, 0:1], in1=scratch[:, 1:2],
                    op0=ALU.mult, op1=ALU.subtract)
                nc.scalar.activation(scratch[:, 3:4], scratch[:, 2:3],
                                     AF.Sqrt, scale=-1.0, bias=eps_t[:])
                nc.vector.reciprocal(scratch[:, 4:5], scratch[:, 3:4])
                nc.vector.tensor_tensor(out=cf[:, 0:1],
                                        in0=vecs[:, gcol:gcol + 1],
                                        in1=scratch[:, 4:5], op=ALU.mult)
                nc.vector.scalar_tensor_tensor(
                    out=cf[:, 1:2], in0=scratch[:, 0:1],
                    scalar=cf[:, 0:1], in1=vecs[:, bcol:bcol + 1],
                    op0=ALU.mult, op1=ALU.subtract)
                nc.vector.tensor_scalar(out=cf[:, 1:2], in0=cf[:, 1:2],
                                        scalar1=-1.0, scalar2=None,
                                        op0=ALU.mult)

            stats_allreduce(0, ar_in, ar_out, cf1, 1, 2)

            # ------------- P3b: bn1+relu -> conv2 -> BN2 stats -------------
            # gates buffers are freed by now: give conv2 a 3-deep pad ring
            ypp_cm = tc.tile_pool(name="ypadp", bufs=1)
            ypp = ypp_cm.__enter__()
            YSLOT = 3
            ypad = ypp.tile([128, YSLOT, GB, 10, 9], BF16, tag="ypad")
            nc.vector.memset(ypad[:], 0)
            ypad_f = ypad[:].rearrange("p s b r w -> p s (b r w)")
            for (gi, (b0, nbs)) in enumerate(groups):
                slot = gi % YSLOT
                ful = all(nb == TILE_B for nb in nbs)
                if ful:
                    nc.scalar.activation(
                        ypad[0:64, slot, :, 1:9, 1:9],
                        R[:, GROUP_T * gi:GROUP_T * (gi + 1)].rearrange(
                            "p t b (h w) -> p (t b) h w", h=8, w=8),
                        AF.Relu, scale=cf1[:, 0:1], bias=cf1[:, 1:2])
                else:
                    for (t, nb) in enumerate(nbs):
                        nc.scalar.activation(
                            ypad[0:64, slot, TILE_B * t:TILE_B * t + nb,
                                 1:9, 1:9],
                            R[:, GROUP_T * gi + t, 0:nb].rearrange(
                                "p b (h w) -> p b h w", h=8, w=8),
                            AF.Relu, scale=cf1[:, 0:1], bias=cf1[:, 1:2])
                nc.sync.dma_start(ypad_f[64:128, slot, 0:GB * PADSZ - 1],
                                  ypad_f[0:64, slot, 1:GB * PADSZ])
                ps, cmb = conv_group(gi, slot, nbs, lhs2, ypad)
                for (t, nb) in enumerate(nbs):
                    ti = GROUP_T * gi + t
                    rsl = R[:, ti, 0:nb]
                    nc.vector.scalar_tensor_tensor(
                        out=rsl.rearrange("p b (h w) -> p b h w", h=8, w=8),
                        in0=ps[0:64, 512 * t:512 * t + nb * 72].rearrange(
                            "p (b r w) -> p b r w", b=nb, r=8, w=9)
                        [:, :, :, 0:8],
                        scalar=1.0, in1=cmb[0:64, t, 0:nb],
                        op0=ALU.mult, op1=ALU.add,
                        accum_out=stats[:, 2 * NT + ti:2 * NT + ti + 1])
                if ful:
                    ti0 = GROUP_T * gi
                    sqj = prep.tile([C, GB, HW], F32, tag="pre")
                    rfl = R[:, GROUP_T * gi:GROUP_T * (gi + 1)].rearrange(
                        "p t b q -> p (t b q)")
                    if gi % 2 == 1:
                        nc.scalar.activation(
                            sqj[:].rearrange("p b q -> p (b q)"), rfl,
                            AF.Square,
                            accum_out=stats[:, 3 * NT + ti0:3 * NT + ti0 + 1])
                    else:
                        nc.vector.scalar_tensor_tensor(
                            out=sqj[:].rearrange("p b q -> p (b q)"), in0=rfl,
                            scalar=1.0, in1=rfl, op0=ALU.mult, op1=ALU.mult,
                            accum_out=stats[:, 3 * NT + ti0:3 * NT + ti0 + 1])
                else:
                    for (t, nb) in enumerate(nbs):
                        ti = GROUP_T * gi + t
                        rsl = R[:, ti, 0:nb]
                        sqj = prep.tile([C, GB, HW], F32, tag="pre")
                        nc.scalar.activation(
                            sqj[:, 0:nb].rearrange("p b q -> p (b q)"),
                            rsl.rearrange("p b q -> p (b q)"), AF.Square,
                            accum_out=stats[:, 3 * NT + ti:3 * NT + ti + 1])

            stats_allreduce(2, ar2_in, ar2_out, cf2, 3, 4)

            # ---------------- P5: bn2 + residual + relu -> out ----------------
            latep_cm = tc.tile_pool(name="latep", bufs=3)
            latep = latep_cm.__enter__()
            for (gi, (b0, nbs)) in enumerate(groups):
                ns = sum(nbs)
                ful = all(nb == TILE_B for nb in nbs)
                stg = latep.tile([C, GB, 8, 8], F32, tag="stg5")
                nc.sync.dma_start(stg[:, 0:ns], x_src(b0, ns))
                pre = prep.tile([C, GB, HW], F32, tag="pre")
                if ful:
                    rsl = R[:, GROUP_T * gi:GROUP_T * (gi + 1)].rearrange(
                        "p t b q -> p (t b) q")
                    nc.vector.scalar_tensor_tensor(
                        out=pre[:], in0=rsl, scalar=cf2[:, 0:1],
                        in1=stg[:].rearrange("p b h w -> p b (h w)"),
                        op0=ALU.mult, op1=ALU.add)
                else:
                    st = 0
                    for (t, nb) in enumerate(nbs):
                        rsl = R[:, GROUP_T * gi + t, 0:nb]
                        nc.vector.scalar_tensor_tensor(
                            out=pre[:, st:st + nb], in0=rsl,
                            scalar=cf2[:, 0:1],
                            in1=stg[:, st:st + nb].rearrange(
                                "p b h w -> p b (h w)"),
                            op0=ALU.mult, op1=ALU.add)
                        st += nb
                nc.scalar.activation(pre[:, 0:ns], pre[:, 0:ns], AF.Relu,
                                     bias=cf2[:, 1:2])
                nc.sync.dma_start(
                    out_d[b0:b0 + ns].transpose([1, 0, 2, 3]),
                    pre[:, 0:ns].rearrange("p b (h w) -> p b h w", h=8, w=8))

            latep_cm.__exit__(None, None, None)
            ypp_cm.__exit__(None, None, None)
            psb_cm.__exit__(None, None, None)
            psc_cm.__exit__(None, None, None)

    nc.compile()
    return nc


_NC_CACHE = {}


def _get_nc(n_cores, b_loc):
    key = (n_cores, b_loc)
    if key not in _NC_CACHE:
        _NC_CACHE[key] = build_nc(n_cores, b_loc)
    return _NC_CACHE[key]


def kernel(**inputs):
    from concourse.bass_utils import run_bass_kernel_spmd

    x = np.asarray(inputs["x"], dtype=np.float32)
    B = x.shape[0]
    n_cores = 8
    b_loc = B // n_cores
    nc = _get_nc(n_cores, b_loc)

    weight_names = ["conv1_w", "conv2_w", "fc1_w", "fc1_b", "fc2_w", "fc2_b",
                    "bn1_g", "bn1_b", "bn2_g", "bn2_b"]
    in_maps = []
    for c in range(n_cores):
        m = {"x": np.ascontiguousarray(x[c * b_loc:(c + 1) * b_loc])}
        for n in weight_names:
            m[n] = np.asarray(inputs[n], dtype=np.float32)
        in_maps.append(m)
    res = run_bass_kernel_spmd(nc, in_maps, core_ids=list(range(n_cores)))
    out = np.concatenate([res.results[c]["out"] for c in range(n_cores)],
                         axis=0)
    return out.astype(np.float32)
